# revision 5
# baseline (speedup 1.0000x reference)
"""Trainium2 Bass kernel for multi-head causal attention.

Problem: x[8,1024,768], Wq/Wk/Wv[12,768,64] -> out[8,1024,768]
  per (b,h): q=x@Wq_h, k=x@Wk_h, v=x@Wv_h, S=q@k^T/8 (causal),
  out[..,64h:64h+64] = softmax(S)@v

Sharding: data parallel over batch — each of the 8 cores handles one batch
element with all 12 heads. No collectives.

Per-core algorithm (all layouts chosen so matmul contraction = partition dim):
  1. x^T [768,1024] built via 48 PE transposes (128x128 blocks).
  2. V = x@Wv in natural [s,64] layout (fp32r matmuls), stored bf16 with a
     ones column appended -> v_aug [s, 12, 65].
  3. Per head-pair P (heads 2P,2P+1 stacked in partitions 0:64 / 64:128):
     Q^T,K^T [128,1024] = (Wqk chunk).T @ x^T  (fp32r, M=128 = two heads).
     S^T [sk,sq] per sk-chunk j: lhsT=k^T slice (K=64, row-group a*64),
     rhs=q^T; causal-skipped at 512 granularity. Causal mask added as an
     extra accumulating matmul: identity.T @ maskT_slice (bf16) writes -1e30
     to the above-diagonal region. exp via ScalarE (scale=1/8) -> bf16 E^T.
     ctx[sq,65] = sum_j E^T[j,sq-block].T @ v_aug[j,h]  (bf16, fp32 accum);
     col 64 = row sums. Normalize with DVE reciprocal + tensor_scalar_mul.
"""

import os

os.environ.setdefault("MYCRO_LOCAL_CACHE", "1")

import numpy as np

import concourse.bass as bass
import concourse.tile as tile
from concourse import bacc, mybir
from concourse.bass_utils import run_bass_kernel_spmd
from concourse.masks import make_identity

F32 = mybir.dt.float32
F32R = mybir.dt.float32r
BF16 = mybir.dt.bfloat16
Exp = mybir.ActivationFunctionType.Exp

S, D, H, DK = 1024, 768, 12, 64  # seq, d_in, heads, d_head
NC = 6  # d_in chunks of 128
NS = 8  # seq chunks of 128
NEG = -1.0e30


def build_program():
    nc = bacc.Bacc("TRN2", target_bir_lowering=False, debug=False)

    x_d = nc.dram_tensor("x", [S, D], F32, kind="ExternalInput").ap()
    wq_d = nc.dram_tensor("wq", [H, D, DK], F32, kind="ExternalInput").ap()
    wk_d = nc.dram_tensor("wk", [H, D, DK], F32, kind="ExternalInput").ap()
    wv_d = nc.dram_tensor("wv", [H, D, DK], F32, kind="ExternalInput").ap()
    out_d = nc.dram_tensor("out", [S, D], F32, kind="ExternalOutput").ap()

    def r32(ap):
        return ap

    with tile.TileContext(nc) as tc:
        from contextlib import ExitStack

        with ExitStack() as ctx:
            consts = ctx.enter_context(tc.tile_pool(name="consts", bufs=1))
            xT_pool = ctx.enter_context(tc.tile_pool(name="xT", bufs=1))
            w_pool = ctx.enter_context(tc.tile_pool(name="w", bufs=1))
            vaug_pool = ctx.enter_context(tc.tile_pool(name="vaug", bufs=1))
            qkT_pool = ctx.enter_context(tc.tile_pool(name="qkT", bufs=4))
            e_pool = ctx.enter_context(tc.tile_pool(name="E", bufs=20))
            out_pool = ctx.enter_context(tc.tile_pool(name="osb", bufs=1))
            small = ctx.enter_context(tc.tile_pool(name="small", bufs=8))

            ident = consts.tile([128, 128], F32, tag="ident")
            make_identity(nc, ident)
            ident_bf = consts.tile([128, 128], BF16, tag="ident_bf")
            make_identity(nc, ident_bf)
            # maskT[r, z] = NEG if z < 384 + r else 0; slice [384-128m : +W]
            # gives the additive causal mask for a diagonal-region block whose
            # sk-chunk starts 128m columns into the written sq span.
            maskT = consts.tile([128, 896], BF16, tag="maskT")
            nc.gpsimd.memset(maskT, 0.0)
            nc.gpsimd.affine_select(
                out=maskT, in_=maskT,
                compare_op=mybir.AluOpType.is_ge,
                fill=NEG, base=-384, channel_multiplier=-1,
                pattern=[[1, 896]],
            )

            # ---- x^T [768, 1024] as 6 tiles [128, 1024] ----
            xT = [xT_pool.tile([128, S], F32R, tag=f"xT{c}", name=f"xT{c}") for c in range(NC)]
            with tc.tile_pool(name="xsb", bufs=3) as xsb, \
                 tc.tile_pool(name="xtp", bufs=4, space="PSUM") as xtp:
                for i in range(NS):
                    xs = xsb.tile([128, D], F32, tag="xs")
                    nc.sync.dma_start(out=xs, in_=x_d[128 * i:128 * (i + 1), :])
                    for c in range(NC):
                        tp = xtp.tile([128, 128], F32, tag="tp")
                        nc.tensor.transpose(tp, xs[:, 128 * c:128 * (c + 1)], ident)
                        nc.scalar.copy(out=xT[c][:, 128 * i:128 * (i + 1)], in_=tp)

            # ---- weights: Wqk [128, 1536] (q heads 0-11 then k heads 0-11),
            #      Wv [128, 768], per d-chunk ----
            wqk = [w_pool.tile([128, 2 * H * DK], F32R, tag=f"wqk{c}", name=f"wqk{c}") for c in range(NC)]
            wv = [w_pool.tile([128, H * DK], F32R, tag=f"wv{c}", name=f"wv{c}") for c in range(NC)]
            for c in range(NC):
                for kind, src in ((0, wq_d), (1, wk_d)):
                    src_ap = bass.AP(
                        tensor=src.tensor,
                        offset=128 * c * DK,
                        ap=[[DK, 128], [D * DK, H], [1, DK]],
                    )
                    dst = wqk[c][:, kind * H * DK:(kind + 1) * H * DK]
                    nc.gpsimd.dma_start(
                        out=dst.rearrange("p (h k) -> p h k", k=DK), in_=src_ap)
                src_ap = bass.AP(
                    tensor=wv_d.tensor,
                    offset=128 * c * DK,
                    ap=[[DK, 128], [D * DK, H], [1, DK]],
                )
                nc.gpsimd.dma_start(
                    out=wv[c].rearrange("p (h k) -> p h k", k=DK), in_=src_ap)

            # ---- V projection: v_aug[j] [128, 12, 65] bf16, col 64 = 1.0 ----
            vaug = [vaug_pool.tile([128, H, DK + 1], BF16, tag=f"vaug{j}", name=f"vaug{j}") for j in range(NS)]
            with tc.tile_pool(name="vps", bufs=2, space="PSUM") as vps_pool:
                for j in range(NS):
                    vps = vps_pool.tile([128, H * DK], F32, tag="vps")
                    for c in range(NC):
                        lhs = r32(xT[c][:, 128 * j:128 * (j + 1)])
                        nc.tensor.matmul(vps[:, 0:512], lhs, r32(wv[c][:, 0:512]),
                                         start=(c == 0), stop=(c == NC - 1))
                        nc.tensor.matmul(vps[:, 512:768], lhs, r32(wv[c][:, 512:768]),
                                         start=(c == 0), stop=(c == NC - 1))
                    nc.vector.memset(vaug[j], 1.0)
                    nc.vector.tensor_copy(
                        out=vaug[j][:, :, 0:DK],
                        in_=vps.rearrange("p (h k) -> p h k", k=DK))

            # ---- per-pair: QK projection, scores, softmax, ctx ----
            out_sb = [out_pool.tile([128, H * DK], F32, tag=f"osb{i}", name=f"osb{i}") for i in range(NS)]
            with tc.tile_pool(name="qkps", bufs=2, space="PSUM") as qkps, \
                 tc.tile_pool(name="sps", bufs=2, space="PSUM") as sps_pool, \
                 tc.tile_pool(name="cps", bufs=2, space="PSUM") as cps_pool:
                for P in range(H // 2):
                    qT = qkT_pool.tile([128, S], F32R, tag="qkT")
                    kT = qkT_pool.tile([128, S], F32R, tag="qkT")
                    for kind, dst in ((0, qT), (1, kT)):
                        for t in range(2):
                            ps = qkps.tile([128, 512], F32, tag="qkps")
                            for c in range(NC):
                                nc.tensor.matmul(
                                    ps,
                                    r32(wqk[c][:, kind * H * DK + 128 * P:
                                               kind * H * DK + 128 * (P + 1)]),
                                    r32(xT[c][:, 512 * t:512 * (t + 1)]),
                                    start=(c == 0), stop=(c == NC - 1))
                            nc.vector.tensor_copy(
                                out=dst[:, 512 * t:512 * (t + 1)], in_=ps)

                    # scores + exp, per sk-chunk j; heads a=0,1 interleaved
                    E = {}
                    for j in range(NS):
                        t0 = j // 4
                        span = S - 512 * t0
                        sp = [sps_pool.tile([128, 1024], F32, tag="sps", name="sps") for _ in range(2)]
                        for t in range(t0, 2):
                            for a in range(2):
                                nc.tensor.matmul(
                                    sp[a][:, 512 * (t - t0):512 * (t - t0) + 512],
                                    r32(kT[64 * a:64 * (a + 1), 128 * j:128 * (j + 1)]),
                                    r32(qT[64 * a:64 * (a + 1), 512 * t:512 * (t + 1)]),
                                    start=True, stop=(t != t0),
                                    tile_position=(64 * a, 0))
                        m = j % 4
                        wid = 128 * (m + 1)
                        for a in range(2):
                            nc.tensor.matmul(
                                sp[a][:, 0:wid],
                                ident_bf,
                                maskT[:, 384 - 128 * m:384 - 128 * m + wid],
                                start=False, stop=True)
                            e = e_pool.tile([128, 1024], BF16, tag="E")
                            nc.scalar.activation(
                                out=e[:, 0:span], in_=sp[a][:, 0:span],
                                func=Exp, scale=0.125)
                            E[(a, j)] = e

                    # ctx + normalize
                    for a in range(2):
                        h = 2 * P + a
                        for i in range(NS):
                            cps = cps_pool.tile([128, DK + 1], F32, tag="cps")
                            for j in range(i + 1):
                                t0 = j // 4
                                nc.tensor.matmul(
                                    cps,
                                    E[(a, j)][:, 128 * i - 512 * t0:
                                              128 * i - 512 * t0 + 128],
                                    vaug[j][:, h, :],
                                    start=(j == 0), stop=(j == i))
                            rec = small.tile([128, 1], F32, tag="rec")
                            nc.vector.reciprocal(rec, cps[:, DK:DK + 1])
                            nc.vector.tensor_scalar_mul(
                                out_sb[i][:, DK * h:DK * (h + 1)],
                                cps[:, 0:DK], rec)

            for i in range(NS):
                nc.sync.dma_start(
                    out=out_d[128 * i:128 * (i + 1), :], in_=out_sb[i])

    nc.compile()
    return nc


_nc_cache = None


def get_program():
    global _nc_cache
    if _nc_cache is None:
        _nc_cache = build_program()
    return _nc_cache


def run(x, Wq, Wk, Wv, trace=False):
    nc = get_program()
    B = x.shape[0]
    in_maps = [
        {
            "x": np.ascontiguousarray(x[b], dtype=np.float32),
            "wq": np.ascontiguousarray(Wq, dtype=np.float32),
            "wk": np.ascontiguousarray(Wk, dtype=np.float32),
            "wv": np.ascontiguousarray(Wv, dtype=np.float32),
        }
        for b in range(B)
    ]
    res = run_bass_kernel_spmd(nc, in_maps, list(range(B)), trace=trace)
    out = np.stack([res.results[b]["out"] for b in range(B)], axis=0)
    return out, res


def kernel(x, Wq, Wk, Wv):
    x = np.asarray(x)
    out, _ = run(np.asarray(x), np.asarray(Wq), np.asarray(Wk), np.asarray(Wv))
    return out.astype(np.float32)
